# revision 14
# baseline (speedup 1.0000x reference)
"""Trainium2 Bass kernel for BertAlibiUnpadSelfAttention.

Problem shapes (hardcoded): B=2, S=2048, H=12, D=64, DIM=768.
Reference computation:
    qkv = hidden @ Wqkv_w.T + Wqkv_b            # (4096, 2304)
    pad via indices (a permutation -> pure row shuffle)
    q,k,v = split/reshape -> (b, h, s, d)
    scores = q @ k.T / sqrt(64) + bias          # bias dense (2,12,2048,2048)
    attn = softmax(scores) @ v -> (4096, 768), unpad via indices

Sharding: 24 (batch, head) pairs -> 3 per core across 8 cores. Each core
computes its own slice of the QKV projection (disjoint columns/rows -> no
redundant FLOPs) and full attention for its 3 heads.

Device kernel layout (v6 - evac rebalance + Schraudolph split):
  - qT/kT computed in [d, s] layout directly; scores computed TRANSPOSED
    (scoresT[sk, sq]) so the softmax reduction is done by the PV matmul
    itself via an appended ones-column on V.
  - QK matmuls (K=64) are packed two-per-PE via 64x128 row tiling; heads
    0/1 run in phase A, head 2 paired with itself in phase B.
  - The evacuation chain (exp + bias product) was the v5 bottleneck:
    ScalarE exp alone is (1024+352)/1.2GHz = 1.15us per [128,1024] tile x
    96 tiles = 110us.  v6 splits the work three ways:
      * ~2/3 of tiles: ScalarE exp -> DVE (or GpSimd) multiply by exp(bias)
        shipped from host as fp16.
      * ~1/3 of tiles ("Schraudolph tiles"): ONE DVE scalar_tensor_tensor
        op computes pt = bits_fp16(int16(scores + bias*1024/ln2 - 59 +
        15360.5)), i.e. a mean-centered Schraudolph exp2 with the bias add
        FUSED - no ScalarE, no separate multiply.  Scores arrive pre-scaled
        by 1024/ln2 (baked into Wq), so the PSUM value IS log2(p)*1024.
      * a slice of the multiplies go to the otherwise-idle GpSimd engine.
  - The QKV projection phase is overlapped into phase A: only k0 + the
    first q0 chunk are computed up front; remaining q0 chunks, the packed
    q1k1 chunk series and the V projection packets are injected into
    phase-A PE idle slots.
  - q1/k1 (head 2) projections are PACKED into one M=128 matmul series
    (q1 on PSUM rows 0:64, k1 on rows 64:128) - halves their PE cost.
  - Projection bias adds moved to ScalarE (activation Identity with a
    per-partition bias AP); output evacuations split between ScalarE and
    VectorE; memsets on GpSimd.
  - Final normalize (divide by sums) + transpose back to [s, d] + V-bias
    add happen on the host (tiny: 3x65x2048 per core).
"""

import math
import numpy as np

B, S, H, D = 2, 2048, 12, 64
DIM = H * D            # 768
TOTAL = B * S          # 4096
HPC = 3                # heads per core
N_CORES = 8
KT = DIM // 128        # 6 k-tiles of 128
SQC = S // 512         # 4 free-dim chunks of 512
SKT = S // 128         # 16 sk tiles of 128
VST = HPC * 65         # vp cols per st block: [h0 64 + one | h1 ... | h2 ...]

A_EXP = 1024.0 / math.log(2.0)   # q-side pre-scale (matches exp affine)
C_SCH = 59.0                     # Schraudolph mean-centering (1024*log2 E[rho])
SCH_OFF = 15360.5                # fp16 exponent bias<<10, +0.5 for floor conv
LAG = 4                          # PV issue lag (iterations) behind QK/evac
PRE = 6                          # bias DMA prefetch depth (tiles)

NIT_A = 2 * 2 * SKT              # 64 phase-A iterations
NIT_B = 2 * 2 * (SKT // 2)       # 32 phase-B iterations


def _sch_A(i):   # Schraudolph (DVE exp2) tiles, phase A (off: error margin)
    return False


def _sch_B(i):
    return False


def _gmul_A(i):  # exp-path multiplies routed to GpSimd (off: slow sems)
    return False


def _gmul_B(i):
    return False


_CACHE = {}


def _build_nc():
    """Build + compile the per-core Bass module (fp16 operands, fp32 PSUM)."""
    from concourse import bacc, mybir, tile
    from concourse.alu_op_type import AluOpType

    f32 = mybir.dt.float32
    f16 = mybir.dt.float16
    i16 = mybir.dt.int16

    nc = bacc.Bacc("TRN2", target_bir_lowering=False, debug=False)

    hT = nc.dram_tensor("hT", (DIM, S), f16, kind="ExternalInput")
    # weights packed per k-tile side by side: wq/wk carry head0+head1 cols
    # (128); wqk1 carries [head2-q (64) | head2-k (64)]; wv all 192.
    wq = nc.dram_tensor("wq", (128, KT * 128), f16, kind="ExternalInput")
    wk = nc.dram_tensor("wk", (128, KT * 128), f16, kind="ExternalInput")
    wqk1 = nc.dram_tensor("wqk1", (128, KT * 128), f16, kind="ExternalInput")
    wv = nc.dram_tensor("wv", (128, KT * HPC * D), f16, kind="ExternalInput")
    # projection bias vectors:
    # col0 = bq[0:128]; col1 rows 0:64 = bq[128:192];
    # col2 = bk[0:128]; col3 rows 64:128 = bk[128:192]
    bqk = nc.dram_tensor("bqk", (128, 4), f32, kind="ExternalInput")
    # transformed bias as fp16 bits, pre-interleaved for paired evacuation:
    # exp(b) for ScalarE tiles, b*1024/ln2 - C_SCH for Schraudolph tiles.
    biasA = nc.dram_tensor("biasA", (S, 2 * S), i16, kind="ExternalInput")
    biasB = nc.dram_tensor("biasB", (S // 2, 2 * S), i16, kind="ExternalInput")
    out = nc.dram_tensor("out", (HPC, D + 1, S), f32, kind="ExternalOutput")

    EXP = mybir.ActivationFunctionType.Exp
    IDENT = mybir.ActivationFunctionType.Identity
    ADD = AluOpType.add
    MUL = AluOpType.mult

    with tile.TileContext(nc) as tc:
        with (
            tc.tile_pool(name="const", bufs=1) as constp,
            tc.tile_pool(name="bias", bufs=PRE + 3) as biasp,
            tc.tile_pool(name="pt", bufs=LAG + 4) as ptp,
            tc.tile_pool(name="ot", bufs=4) as otp,
            tc.tile_pool(name="ps", bufs=2, space="PSUM") as psp,
            tc.tile_pool(name="po", bufs=3, space="PSUM") as pop,
            tc.tile_pool(name="px", bufs=1, space="PSUM") as pxp,
        ):
            # ---- load persistent inputs ----
            # hT arrives in column chunks so the first k0/q0 projection
            # chunk can start after ~0.75MB instead of the full 3MB.
            ht = [constp.tile([128, S], f16, tag=f"ht{i}", name=f"ht{i}")
                  for i in range(KT)]
            for c in range(SQC):
                for i in range(KT):
                    nc.sync.dma_start(
                        ht[i][:, c * 512:(c + 1) * 512],
                        hT[i * 128:(i + 1) * 128, c * 512:(c + 1) * 512])

            wq_all = constp.tile([128, KT * 128], f16, tag="wqa")
            wk_all = constp.tile([128, KT * 128], f16, tag="wka")
            wqk1_all = constp.tile([128, KT * 128], f16, tag="wqk1a")
            wv_all = constp.tile([128, KT * HPC * D], f16, tag="wva")
            bqk_sb = constp.tile([128, 4], f32, tag="bqk")
            nc.scalar.dma_start(bqk_sb[:], bqk[:, :])
            nc.scalar.dma_start(wq_all[:], wq[:, :])
            nc.scalar.dma_start(wk_all[:], wk[:, :])
            nc.scalar.dma_start(wqk1_all[:], wqk1[:, :])
            nc.scalar.dma_start(wv_all[:], wv[:, :])
            wq_sb = [wq_all[:, i * 128:(i + 1) * 128] for i in range(KT)]
            wk_sb = [wk_all[:, i * 128:(i + 1) * 128] for i in range(KT)]
            wqk1_sb = [wqk1_all[:, i * 128:(i + 1) * 128] for i in range(KT)]
            wv_sb = [wv_all[:, i * HPC * D:(i + 1) * HPC * D] for i in range(KT)]
            bq_sb = bqk_sb[:, 0:1]
            bq2_sb = bqk_sb[0:64, 1:2]
            bk_sb = bqk_sb[:, 2:3]
            bk2_sb = bqk_sb[64:128, 3:4]
            # Q/K in [d, s] layout: heads 0,1 in q0/k0 (partitions 0-63 /
            # 64-127); head 2 on partitions 0-63 of q1 (dup'd to 64-127) and
            # partitions 64-127 of k1 (dup'd to 0-63) for row-tiled pairing.
            q0 = constp.tile([128, S], f16, tag="q0")
            q1 = constp.tile([128, S], f16, tag="q1")
            k0 = constp.tile([128, S], f16, tag="k0")
            k1 = constp.tile([128, S], f16, tag="k1")
            # V' blocks per st: [h0 d0..63, one, h1 d0..63, one, h2 ...];
            # only the ones-columns need the memset (vproj writes the rest).
            vp = constp.tile([128, SKT * VST], f16, tag="vp")
            warm = constp.tile([128, 512], f16, tag="warm")
            nc.gpsimd.memset(warm[:], 0.5)
            ones_view = vp[:].rearrange(
                "p (st j d) -> p st j d", st=SKT, j=HPC)[:, :, :, 64:65]
            nc.gpsimd.memset(ones_view, 1.0)

            # ---- projection packets (PE + ScalarE/VectorE evac) ----
            def q0k0_chunk(dst, wsb, bsb, c, engine):
                ps = pxp.tile([128, 512], f32, tag="px", name=f"pj{c}")
                for i in range(KT):
                    nc.tensor.matmul(
                        ps[:], wsb[i],
                        ht[i][:, c * 512:(c + 1) * 512],
                        start=(i == 0), stop=(i == KT - 1))
                if engine == "v":
                    nc.vector.tensor_scalar_add(
                        dst[:, c * 512:(c + 1) * 512], ps[:], bsb)
                else:
                    nc.scalar.activation(
                        dst[:, c * 512:(c + 1) * 512], ps[:], IDENT,
                        bias=bsb, scale=1.0)

            def q1k1_chunk(c, engine):
                # packed: PSUM rows 0:64 = head2 q, rows 64:128 = head2 k
                ps = pxp.tile([128, 512], f32, tag="px", name=f"pqk1{c}")
                for i in range(KT):
                    nc.tensor.matmul(
                        ps[:], wqk1_sb[i],
                        ht[i][:, c * 512:(c + 1) * 512],
                        start=(i == 0), stop=(i == KT - 1))
                if engine == "v":
                    nc.vector.tensor_scalar_add(
                        q1[0:64, c * 512:(c + 1) * 512], ps[0:64, :], bq2_sb)
                    nc.vector.tensor_scalar_add(
                        k1[64:128, c * 512:(c + 1) * 512], ps[64:128, :], bk2_sb)
                else:
                    nc.scalar.activation(
                        q1[0:64, c * 512:(c + 1) * 512], ps[0:64, :], IDENT,
                        bias=bq2_sb, scale=1.0)
                    nc.scalar.activation(
                        k1[64:128, c * 512:(c + 1) * 512], ps[64:128, :], IDENT,
                        bias=bk2_sb, scale=1.0)

            def v_proj2(st):
                # V projection for TWO st tiles in one packet: 12 matmuls,
                # one PSUM tile, ONE evacuation copy (halves the op count
                # and keeps px workable at bufs=1).
                psv = pxp.tile([128, 2 * HPC * D], f32, tag="px", name="psVx")
                for s in range(2):
                    for i in range(KT):
                        nc.tensor.matmul(
                            psv[:, s * HPC * D:(s + 1) * HPC * D],
                            ht[i][:, (st + s) * 128:(st + s + 1) * 128],
                            wv_sb[i],
                            start=(i == 0), stop=(i == KT - 1))
                nc.vector.tensor_copy(
                    vp[:, st * VST:(st + 2) * VST].rearrange(
                        "p (s j d) -> p s j d", s=2, j=HPC)[:, :, :, 0:D],
                    psv[:].rearrange("p (s j d) -> p s j d", s=2, j=HPC))

            # ---- prologue: warmup + k0 + first q0/q1k1 chunks ----
            # PE warmup during the input-DMA window keeps the HAM clock
            # gate at 8/8 so the whole kernel runs at 2.4 GHz.
            for w in range(17):
                wps = pxp.tile([128, 512], f32, tag="px", name=f"wu{w}")
                nc.tensor.matmul(wps[:], warm[:, 0:128], warm[:],
                                 start=True, stop=True)
            # minimal prologue: k0/q0 first chunks only; the rest is
            # injected into phase-A PE idle slots (see injected()).
            q0k0_chunk(k0, wk_sb, bk_sb, 0, "v")
            q0k0_chunk(q0, wq_sb, bq_sb, 0, "v")
            v_proj2(0)

            def sqoff(cp, c2):
                return cp * 1024 + c2 * 512

            # ---------- phase A: heads 0 and 1 ----------
            bt_tiles = {}
            pt_tiles = {}
            po_t = {}

            def bias_dma_A(i):
                if i >= NIT_A:
                    return
                cpc2, st = divmod(i, SKT)
                cp, c2 = divmod(cpc2, 2)
                bt = biasp.tile([128, 1024], i16, tag="bt", name=f"btA{i}")
                col = cp * 2048 + c2 * 1024
                nc.sync.dma_start(
                    bt[:], biasA[st * 128:(st + 1) * 128, col:col + 1024])
                bt_tiles[i] = bt

            def qk_evac_A(i):
                cpc2, st = divmod(i, SKT)
                cp, c2 = divmod(cpc2, 2)
                bias_dma_A(i + PRE)
                sq = sqoff(cp, c2)
                ps = psp.tile([128, 1024], f32, tag="ps", name=f"psA2_{i}")
                pt = ptp.tile([128, 1024], f16, tag="pt", name=f"ptA{i}")
                pt_tiles[i] = pt
                nc.tensor.matmul(
                    ps[:, 0:512], k0[0:64, st * 128:(st + 1) * 128],
                    q0[0:64, sq:sq + 512], start=True, stop=True)
                nc.tensor.matmul(
                    ps[:, 512:1024], k0[64:128, st * 128:(st + 1) * 128],
                    q0[64:128, sq:sq + 512], start=True, stop=True)
                bt = bt_tiles.pop(i)
                if _sch_A(i):
                    nc.vector.scalar_tensor_tensor(
                        pt[:].bitcast(i16), ps[:], SCH_OFF,
                        bt[:].bitcast(f16), ADD, ADD)
                else:
                    nc.scalar.activation(pt[:], ps[:], EXP, scale=1.0 / A_EXP)
                    if _gmul_A(i):
                        nc.gpsimd.tensor_tensor(
                            pt[:], pt[:], bt[:].bitcast(f16), MUL)
                    else:
                        nc.vector.tensor_mul(pt[:], pt[:], bt[:].bitcast(f16))

            def pv_A(i):
                cpc2, st = divmod(i, SKT)
                cp, c2 = divmod(cpc2, 2)
                if st == 0:
                    po_t[cpc2] = [
                        pop.tile([D + 1, 512], f32, tag="po",
                                 name=f"poA{cpc2}_{h}")
                        for h in range(2)]
                po = po_t[cpc2]
                pt = pt_tiles.pop(i)
                for h in range(2):
                    nc.tensor.matmul(
                        po[h][:],
                        vp[:, st * VST + h * 65: st * VST + h * 65 + D + 1],
                        pt[:, h * 512:(h + 1) * 512],
                        start=(st == 0), stop=(st == SKT - 1))
                if st == SKT - 1:
                    for h in range(2):
                        ot = otp.tile([D + 1, 512], f32, tag="ot", name="ot")
                        nc.vector.tensor_copy(ot[:], po[h][:])
                        nc.sync.dma_start(
                            out[h, :, sqoff(cp, c2):sqoff(cp, c2) + 512],
                            ot[:])

            # phase-A injected packet schedule (group index -> packets).
            # k0 chunk c is needed by iteration 4c (group 2c); q0 chunk c by
            # iteration 16c (group 8c); q1/k1 only by phase B.
            def injected(g):
                if g <= 6:
                    v_proj2(2 * g + 2)
                if g in (0, 1, 2):
                    q0k0_chunk(k0, wk_sb, bk_sb, g + 1, "v")
                elif g in (3, 5, 7):
                    q0k0_chunk(q0, wq_sb, bq_sb, (g - 1) // 2, "v")
                elif g in (9, 11, 13, 15):
                    q1k1_chunk((g - 9) // 2, "v")
                elif g == 17:
                    # duplicate head-2 q/k halves for row-tiled self-pairing
                    nc.scalar.dma_start(q1[64:128, :], q1[0:64, :])
                    nc.scalar.dma_start(k1[0:64, :], k1[64:128, :])

            for i in range(PRE):
                bias_dma_A(i)
            for i0 in range(0, NIT_A + LAG, 2):
                for di in range(2):
                    if i0 + di < NIT_A:
                        qk_evac_A(i0 + di)
                injected(i0 // 2)
                for di in range(2):
                    ip = i0 + di - LAG
                    if 0 <= ip < NIT_A:
                        pv_A(ip)

            # ---------- phase B: head 2 (paired with itself) ----------
            bt2_tiles = {}
            pt2_tiles = {}
            po2_t = {}

            def bias_dma_B(i):
                if i >= NIT_B:
                    return
                cpc2, stp = divmod(i, SKT // 2)
                cp, c2 = divmod(cpc2, 2)
                bt = biasp.tile([128, 1024], i16, tag="bt", name=f"btB{i}")
                col = cp * 2048 + c2 * 1024
                nc.sync.dma_start(
                    bt[:], biasB[stp * 128:(stp + 1) * 128, col:col + 1024])
                bt2_tiles[i] = bt

            def qk_evac_B(i):
                cpc2, stp = divmod(i, SKT // 2)
                cp, c2 = divmod(cpc2, 2)
                bias_dma_B(i + PRE)
                sq = sqoff(cp, c2)
                st0, st1 = 2 * stp, 2 * stp + 1
                ps = psp.tile([128, 1024], f32, tag="ps", name=f"psB2_{i}")
                pt = ptp.tile([128, 1024], f16, tag="pt", name=f"ptB{i}")
                pt2_tiles[i] = pt
                nc.tensor.matmul(
                    ps[:, 0:512], k1[0:64, st0 * 128:(st0 + 1) * 128],
                    q1[0:64, sq:sq + 512], start=True, stop=True)
                nc.tensor.matmul(
                    ps[:, 512:1024], k1[64:128, st1 * 128:(st1 + 1) * 128],
                    q1[64:128, sq:sq + 512], start=True, stop=True)
                bt = bt2_tiles.pop(i)
                if _sch_B(i):
                    nc.vector.scalar_tensor_tensor(
                        pt[:].bitcast(i16), ps[:], SCH_OFF,
                        bt[:].bitcast(f16), ADD, ADD)
                else:
                    nc.scalar.activation(pt[:], ps[:], EXP, scale=1.0 / A_EXP)
                    if _gmul_B(i):
                        nc.gpsimd.tensor_tensor(
                            pt[:], pt[:], bt[:].bitcast(f16), MUL)
                    else:
                        nc.vector.tensor_mul(pt[:], pt[:], bt[:].bitcast(f16))

            def pv_B(i):
                cpc2, stp = divmod(i, SKT // 2)
                cp, c2 = divmod(cpc2, 2)
                if stp == 0:
                    po2_t[cpc2] = pop.tile(
                        [D + 1, 512], f32, tag="po", name=f"poB{cpc2}")
                po = po2_t[cpc2]
                pt = pt2_tiles.pop(i)
                for par in range(2):
                    st = 2 * stp + par
                    nc.tensor.matmul(
                        po[:],
                        vp[:, st * VST + 2 * 65: st * VST + 2 * 65 + D + 1],
                        pt[:, par * 512:(par + 1) * 512],
                        start=(stp == 0 and par == 0),
                        stop=(stp == SKT // 2 - 1 and par == 1))
                if stp == SKT // 2 - 1:
                    ot = otp.tile([D + 1, 512], f32, tag="ot", name="ot")
                    nc.vector.tensor_copy(ot[:], po[:])
                    nc.scalar.dma_start(
                        out[2, :, sqoff(cp, c2):sqoff(cp, c2) + 512],
                        ot[:])

            for i in range(PRE):
                bias_dma_B(i)
            for i0 in range(0, NIT_B + LAG, 2):
                for di in range(2):
                    if i0 + di < NIT_B:
                        qk_evac_B(i0 + di)
                for di in range(2):
                    ip = i0 + di - LAG
                    if 0 <= ip < NIT_B:
                        pv_B(ip)

    nc.compile()
    return nc


def _get_nc(variant=None):
    if "nc" not in _CACHE:
        _CACHE["nc"] = _build_nc()
    return _CACHE["nc"]


def _transform_bias(bias_c):
    """Per-tile transformed bias for one core: exp(b) for ScalarE tiles,
    b*1024/ln2 - C_SCH (Schraudolph) for DVE tiles.  bias_c: [3, sk, sq]."""
    tb = np.empty((HPC, S, S), dtype=np.float16)
    for cpc2 in range(4):
        cp, c2 = divmod(cpc2, 2)
        cs = slice(cp * 1024 + c2 * 512, cp * 1024 + c2 * 512 + 512)
        for st in range(SKT):
            i = cpc2 * SKT + st
            rs = slice(st * 128, (st + 1) * 128)
            for h in (0, 1):
                src = bias_c[h][rs, cs]
                if _sch_A(i):
                    tb[h][rs, cs] = (A_EXP * src - C_SCH).astype(np.float16)
                else:
                    tb[h][rs, cs] = np.exp(src).astype(np.float16)
        for stp in range(SKT // 2):
            i = cpc2 * (SKT // 2) + stp
            rs = slice(stp * 256, (stp + 1) * 256)
            src = bias_c[2][rs, cs]
            if _sch_B(i):
                tb[2][rs, cs] = (A_EXP * src - C_SCH).astype(np.float16)
            else:
                tb[2][rs, cs] = np.exp(src).astype(np.float16)
    return tb


def _make_in_maps(hidden_states, Wqkv_w, Wqkv_b, bias, indices, variant=None):
    hidden_states = np.asarray(hidden_states, dtype=np.float32)
    Wqkv_w = np.asarray(Wqkv_w, dtype=np.float32)
    Wqkv_b = np.asarray(Wqkv_b, dtype=np.float32)
    bias = np.asarray(bias, dtype=np.float32)
    indices = np.asarray(indices, dtype=np.int64)

    qscale = np.float32(A_EXP / math.sqrt(D))
    padded = np.zeros((TOTAL, DIM), dtype=np.float32)
    padded[indices] = hidden_states

    Wq, Wk, Wv = Wqkv_w[0:DIM], Wqkv_w[DIM:2 * DIM], Wqkv_w[2 * DIM:3 * DIM]
    bq_full = Wqkv_b[0:DIM] * qscale
    bk_full = Wqkv_b[DIM:2 * DIM]

    def pack_w(WT):  # [768, C] -> [128, 6*C]
        C = WT.shape[1]
        return np.ascontiguousarray(
            WT.reshape(KT, 128, C).transpose(1, 0, 2).reshape(128, KT * C))

    in_maps = []
    for cidx in range(N_CORES):
        b = cidx // 4
        h0 = (cidx % 4) * HPC
        r = slice(h0 * D, (h0 + HPC) * D)
        bias_c = np.ascontiguousarray(bias[b, h0:h0 + HPC].transpose(0, 2, 1))
        tb = _transform_bias(bias_c).view(np.int16)  # [3, sk, sq]
        # biasA: [sk 2048, (cp 2, c2 2, h 2, x 512)]
        bA = tb[0:2].reshape(2, S, 2, 2, 512).transpose(1, 2, 3, 0, 4)
        bA = np.ascontiguousarray(bA.reshape(S, 2 * S))
        # biasB: [stp*128+p, (cp 2, c2 2, par 2, x 512)]
        bB = tb[2].reshape(8, 2, 128, 2, 2, 512).transpose(0, 2, 3, 4, 1, 5)
        bB = np.ascontiguousarray(bB.reshape(S // 2, 2 * S))
        bqk_arr = np.zeros((128, 4), dtype=np.float32)
        bqk_arr[:, 0] = bq_full[r][0:128]
        bqk_arr[0:64, 1] = bq_full[r][128:192]
        bqk_arr[:, 2] = bk_full[r][0:128]
        bqk_arr[64:128, 3] = bk_full[r][128:192]
        WqT = Wq[r].T * qscale
        WkT = Wk[r].T
        in_maps.append({
            "hT": padded[b * S:(b + 1) * S].T.astype(np.float16),
            "wq": pack_w(WqT[:, 0:128].astype(np.float16)),
            "wk": pack_w(WkT[:, 0:128].astype(np.float16)),
            "wqk1": pack_w(np.concatenate(
                [WqT[:, 128:192], WkT[:, 128:192]], axis=1).astype(np.float16)),
            "wv": pack_w(Wv[r].T.astype(np.float16)),
            "bqk": bqk_arr,
            "biasA": bA,
            "biasB": bB,
        })
    return in_maps


def _assemble(results, Wqkv_b, indices):
    Wqkv_b = np.asarray(Wqkv_b, dtype=np.float32)
    indices = np.asarray(indices, dtype=np.int64)
    bv = Wqkv_b[2 * DIM:3 * DIM]
    out_full = np.empty((TOTAL, DIM), dtype=np.float32)
    for c in range(N_CORES):
        b = c // 4
        h0 = (c % 4) * HPC
        o = np.asarray(results[c]["out"], dtype=np.float32)  # (3, 65, 2048)
        for j in range(HPC):
            h = h0 + j
            att = (o[j, :D] / o[j, D]).T + bv[h * D:(h + 1) * D]
            out_full[b * S:(b + 1) * S, h * D:(h + 1) * D] = att
    return out_full[indices]


VARIANT = "v6"


def kernel(hidden_states, Wqkv_w, Wqkv_b, bias, slopes, cu_seqlens, indices,
           attn_mask, max_seqlen, **_unused):
    from concourse.bass_utils import run_bass_kernel_spmd

    nc = _get_nc()
    in_maps = _make_in_maps(hidden_states, Wqkv_w, Wqkv_b, bias, indices)
    res = run_bass_kernel_spmd(nc, in_maps, list(range(N_CORES)))
    return _assemble(res.results, Wqkv_b, indices)


# revision 17
# speedup vs baseline: 1.0205x; 1.0205x over previous
"""Trainium2 Bass kernel for BertAlibiUnpadSelfAttention.

Problem shapes (hardcoded): B=2, S=2048, H=12, D=64, DIM=768.
Reference computation:
    qkv = hidden @ Wqkv_w.T + Wqkv_b            # (4096, 2304)
    pad via indices (a permutation -> pure row shuffle)
    q,k,v = split/reshape -> (b, h, s, d)
    scores = q @ k.T / sqrt(64) + bias          # bias dense (2,12,2048,2048)
    attn = softmax(scores) @ v -> (4096, 768), unpad via indices

Sharding: 24 (batch, head) pairs -> 3 per core across 8 cores. Each core
computes its own slice of the QKV projection (disjoint columns/rows -> no
redundant FLOPs) and full attention for its 3 heads.

Device kernel layout (v6 - evac rebalance + Schraudolph split):
  - qT/kT computed in [d, s] layout directly; scores computed TRANSPOSED
    (scoresT[sk, sq]) so the softmax reduction is done by the PV matmul
    itself via an appended ones-column on V.
  - QK matmuls (K=64) are packed two-per-PE via 64x128 row tiling; heads
    0/1 run in phase A, head 2 paired with itself in phase B.
  - The evacuation chain (exp + bias product) was the v5 bottleneck:
    ScalarE exp alone is (1024+352)/1.2GHz = 1.15us per [128,1024] tile x
    96 tiles = 110us.  v6 splits the work three ways:
      * ~2/3 of tiles: ScalarE exp -> DVE (or GpSimd) multiply by exp(bias)
        shipped from host as fp16.
      * ~1/3 of tiles ("Schraudolph tiles"): ONE DVE scalar_tensor_tensor
        op computes pt = bits_fp16(int16(scores + bias*1024/ln2 - 59 +
        15360.5)), i.e. a mean-centered Schraudolph exp2 with the bias add
        FUSED - no ScalarE, no separate multiply.  Scores arrive pre-scaled
        by 1024/ln2 (baked into Wq), so the PSUM value IS log2(p)*1024.
      * a slice of the multiplies go to the otherwise-idle GpSimd engine.
  - The QKV projection phase is overlapped into phase A: only k0 + the
    first q0 chunk are computed up front; remaining q0 chunks, the packed
    q1k1 chunk series and the V projection packets are injected into
    phase-A PE idle slots.
  - q1/k1 (head 2) projections are PACKED into one M=128 matmul series
    (q1 on PSUM rows 0:64, k1 on rows 64:128) - halves their PE cost.
  - Projection bias adds moved to ScalarE (activation Identity with a
    per-partition bias AP); output evacuations split between ScalarE and
    VectorE; memsets on GpSimd.
  - Final normalize (divide by sums) + transpose back to [s, d] + V-bias
    add happen on the host (tiny: 3x65x2048 per core).
"""

import math
import numpy as np

B, S, H, D = 2, 2048, 12, 64
DIM = H * D            # 768
TOTAL = B * S          # 4096
HPC = 3                # heads per core
N_CORES = 8
KT = DIM // 128        # 6 k-tiles of 128
SQC = S // 512         # 4 free-dim chunks of 512
SKT = S // 128         # 16 sk tiles of 128
VST = HPC * 65         # vp cols per st block: [h0 64 + one | h1 ... | h2 ...]

A_EXP = 1024.0 / math.log(2.0)   # q-side pre-scale (matches exp affine)
C_SCH = 59.0                     # Schraudolph mean-centering (1024*log2 E[rho])
SCH_OFF = 15360.5                # fp16 exponent bias<<10, +0.5 for floor conv
LAG = 4                          # PV issue lag (iterations) behind QK/evac
PRE = 6                          # bias DMA prefetch depth (tiles)

NIT_A = 2 * 2 * SKT              # 64 phase-A iterations
NIT_B = 2 * 2 * (SKT // 2)       # 32 phase-B iterations


def _sch_A(i):   # Schraudolph (DVE exp2) tiles, phase A (off: error margin)
    return False


def _sch_B(i):
    return False


def _gmul_A(i):  # exp-path multiplies routed to GpSimd (off: slow sems)
    return False


def _gmul_B(i):
    return False


_CACHE = {}


def _build_nc():
    """Build + compile the per-core Bass module (fp16 operands, fp32 PSUM)."""
    from concourse import bacc, mybir, tile
    from concourse.alu_op_type import AluOpType

    f32 = mybir.dt.float32
    f16 = mybir.dt.float16
    i16 = mybir.dt.int16

    nc = bacc.Bacc("TRN2", target_bir_lowering=False, debug=False)

    hT = nc.dram_tensor("hT", (DIM, S), f16, kind="ExternalInput")
    # weights packed per k-tile side by side: wq/wk carry head0+head1 cols
    # (128); wqk1 carries [head2-q (64) | head2-k (64)]; wv all 192.
    wq = nc.dram_tensor("wq", (128, KT * 128), f16, kind="ExternalInput")
    wk = nc.dram_tensor("wk", (128, KT * 128), f16, kind="ExternalInput")
    wqk1 = nc.dram_tensor("wqk1", (128, KT * 128), f16, kind="ExternalInput")
    wv = nc.dram_tensor("wv", (128, KT * HPC * D), f16, kind="ExternalInput")
    # projection bias vectors:
    # col0 = bq[0:128]; col1 rows 0:64 = bq[128:192];
    # col2 = bk[0:128]; col3 rows 64:128 = bk[128:192]
    bqk = nc.dram_tensor("bqk", (128, 4), f32, kind="ExternalInput")
    # transformed bias as fp16 bits, pre-interleaved for paired evacuation:
    # exp(b) for ScalarE tiles, b*1024/ln2 - C_SCH for Schraudolph tiles.
    biasA = nc.dram_tensor("biasA", (S, 2 * S), i16, kind="ExternalInput")
    biasB = nc.dram_tensor("biasB", (S // 2, 2 * S), i16, kind="ExternalInput")
    out = nc.dram_tensor("out", (HPC, D + 1, S), f32, kind="ExternalOutput")

    EXP = mybir.ActivationFunctionType.Exp
    IDENT = mybir.ActivationFunctionType.Identity
    ADD = AluOpType.add
    MUL = AluOpType.mult

    with tile.TileContext(nc) as tc:
        with (
            tc.tile_pool(name="const", bufs=1) as constp,
            tc.tile_pool(name="bias", bufs=PRE + 3) as biasp,
            tc.tile_pool(name="pt", bufs=LAG + 4) as ptp,
            tc.tile_pool(name="ot", bufs=4) as otp,
            tc.tile_pool(name="ps", bufs=2, space="PSUM") as psp,
            tc.tile_pool(name="po", bufs=2, space="PSUM") as pop,
            tc.tile_pool(name="px", bufs=2, space="PSUM") as pxp,
        ):
            # ---- load persistent inputs ----
            # hT arrives in column chunks so the first k0/q0 projection
            # chunk can start after ~0.75MB instead of the full 3MB.
            ht = [constp.tile([128, S], f16, tag=f"ht{i}", name=f"ht{i}")
                  for i in range(KT)]
            for c in range(SQC):
                for i in range(KT):
                    nc.sync.dma_start(
                        ht[i][:, c * 512:(c + 1) * 512],
                        hT[i * 128:(i + 1) * 128, c * 512:(c + 1) * 512])

            wq_all = constp.tile([128, KT * 128], f16, tag="wqa")
            wk_all = constp.tile([128, KT * 128], f16, tag="wka")
            wqk1_all = constp.tile([128, KT * 128], f16, tag="wqk1a")
            wv_all = constp.tile([128, KT * HPC * D], f16, tag="wva")
            bqk_sb = constp.tile([128, 4], f32, tag="bqk")
            nc.scalar.dma_start(bqk_sb[:], bqk[:, :])
            nc.scalar.dma_start(wq_all[:], wq[:, :])
            nc.scalar.dma_start(wk_all[:], wk[:, :])
            nc.scalar.dma_start(wqk1_all[:], wqk1[:, :])
            nc.scalar.dma_start(wv_all[:], wv[:, :])
            wq_sb = [wq_all[:, i * 128:(i + 1) * 128] for i in range(KT)]
            wk_sb = [wk_all[:, i * 128:(i + 1) * 128] for i in range(KT)]
            wqk1_sb = [wqk1_all[:, i * 128:(i + 1) * 128] for i in range(KT)]
            wv_sb = [wv_all[:, i * HPC * D:(i + 1) * HPC * D] for i in range(KT)]
            bq_sb = bqk_sb[:, 0:1]
            bq2_sb = bqk_sb[0:64, 1:2]
            bk_sb = bqk_sb[:, 2:3]
            bk2_sb = bqk_sb[64:128, 3:4]
            # Q/K in [d, s] layout: heads 0,1 in q0/k0 (partitions 0-63 /
            # 64-127); head 2 on partitions 0-63 of q1 (dup'd to 64-127) and
            # partitions 64-127 of k1 (dup'd to 0-63) for row-tiled pairing.
            q0 = constp.tile([128, S], f16, tag="q0")
            q1 = constp.tile([128, S], f16, tag="q1")
            k0 = constp.tile([128, S], f16, tag="k0")
            k1 = constp.tile([128, S], f16, tag="k1")
            # V' blocks per st: [h0 d0..63, one, h1 d0..63, one, h2 ...];
            # only the ones-columns need the memset (vproj writes the rest).
            vp = constp.tile([128, SKT * VST], f16, tag="vp")
            warm = constp.tile([128, 512], f16, tag="warm")
            nc.gpsimd.memset(warm[:], 0.5)
            ones_view = vp[:].rearrange(
                "p (st j d) -> p st j d", st=SKT, j=HPC)[:, :, :, 64:65]
            nc.gpsimd.memset(ones_view, 1.0)

            # ---- projection packets (PE + ScalarE/VectorE evac) ----
            def q0k0_chunk(dst, wsb, bsb, c, engine):
                ps = pxp.tile([128, 512], f32, tag="px", name=f"pj{c}")
                for i in range(KT):
                    nc.tensor.matmul(
                        ps[:], wsb[i],
                        ht[i][:, c * 512:(c + 1) * 512],
                        start=(i == 0), stop=(i == KT - 1))
                if engine == "v":
                    nc.vector.tensor_scalar_add(
                        dst[:, c * 512:(c + 1) * 512], ps[:], bsb)
                else:
                    nc.scalar.activation(
                        dst[:, c * 512:(c + 1) * 512], ps[:], IDENT,
                        bias=bsb, scale=1.0)

            def q1k1_chunk(c, engine):
                # packed: PSUM rows 0:64 = head2 q, rows 64:128 = head2 k
                ps = pxp.tile([128, 512], f32, tag="px", name=f"pqk1{c}")
                for i in range(KT):
                    nc.tensor.matmul(
                        ps[:], wqk1_sb[i],
                        ht[i][:, c * 512:(c + 1) * 512],
                        start=(i == 0), stop=(i == KT - 1))
                if engine == "v":
                    nc.vector.tensor_scalar_add(
                        q1[0:64, c * 512:(c + 1) * 512], ps[0:64, :], bq2_sb)
                    nc.vector.tensor_scalar_add(
                        k1[64:128, c * 512:(c + 1) * 512], ps[64:128, :], bk2_sb)
                else:
                    nc.scalar.activation(
                        q1[0:64, c * 512:(c + 1) * 512], ps[0:64, :], IDENT,
                        bias=bq2_sb, scale=1.0)
                    nc.scalar.activation(
                        k1[64:128, c * 512:(c + 1) * 512], ps[64:128, :], IDENT,
                        bias=bk2_sb, scale=1.0)

            def v_proj2(st):
                # V projection for TWO st tiles in one packet: 12 matmuls,
                # one PSUM tile, ONE evacuation copy (halves the op count
                # and keeps px workable at bufs=1).
                psv = pxp.tile([128, 2 * HPC * D], f32, tag="px", name="psVx")
                for s in range(2):
                    for i in range(KT):
                        nc.tensor.matmul(
                            psv[:, s * HPC * D:(s + 1) * HPC * D],
                            ht[i][:, (st + s) * 128:(st + s + 1) * 128],
                            wv_sb[i],
                            start=(i == 0), stop=(i == KT - 1))
                nc.vector.tensor_copy(
                    vp[:, st * VST:(st + 2) * VST].rearrange(
                        "p (s j d) -> p s j d", s=2, j=HPC)[:, :, :, 0:D],
                    psv[:].rearrange("p (s j d) -> p s j d", s=2, j=HPC))

            # ---- prologue: warmup + k0 + first q0/q1k1 chunks ----
            # PE warmup during the input-DMA window keeps the HAM clock
            # gate at 8/8 so the whole kernel runs at 2.4 GHz.
            for w in range(7):
                wps = pxp.tile([128, 512], f32, tag="px", name=f"wu{w}")
                nc.tensor.matmul(wps[:], warm[:, 0:128], warm[:],
                                 start=True, stop=True)
            # minimal prologue: k0/q0 first chunks only; the rest is
            # injected into phase-A PE idle slots (see injected()).
            q0k0_chunk(k0, wk_sb, bk_sb, 0, "v")
            q0k0_chunk(q0, wq_sb, bq_sb, 0, "v")
            v_proj2(0)

            def sqoff(cp, c2):
                return cp * 1024 + c2 * 512

            # ---------- phase A: heads 0 and 1 ----------
            bt_tiles = {}
            pt_tiles = {}
            po_t = {}

            def bias_dma_A(i):
                if i >= NIT_A:
                    return
                cpc2, st = divmod(i, SKT)
                cp, c2 = divmod(cpc2, 2)
                bt = biasp.tile([128, 1024], i16, tag="bt", name=f"btA{i}")
                col = cp * 2048 + c2 * 1024
                nc.sync.dma_start(
                    bt[:], biasA[st * 128:(st + 1) * 128, col:col + 1024])
                bt_tiles[i] = bt

            def qk_evac_A(i):
                cpc2, st = divmod(i, SKT)
                cp, c2 = divmod(cpc2, 2)
                bias_dma_A(i + PRE)
                sq = sqoff(cp, c2)
                ps = psp.tile([128, 1024], f32, tag="ps", name=f"psA2_{i}")
                pt = ptp.tile([128, 1024], f16, tag="pt", name=f"ptA{i}")
                pt_tiles[i] = pt
                nc.tensor.matmul(
                    ps[:, 0:512], k0[0:64, st * 128:(st + 1) * 128],
                    q0[0:64, sq:sq + 512], start=True, stop=True)
                nc.tensor.matmul(
                    ps[:, 512:1024], k0[64:128, st * 128:(st + 1) * 128],
                    q0[64:128, sq:sq + 512], start=True, stop=True)
                bt = bt_tiles.pop(i)
                if _sch_A(i):
                    nc.vector.scalar_tensor_tensor(
                        pt[:].bitcast(i16), ps[:], SCH_OFF,
                        bt[:].bitcast(f16), ADD, ADD)
                else:
                    nc.scalar.activation(pt[:], ps[:], EXP, scale=1.0 / A_EXP)
                    if _gmul_A(i):
                        nc.gpsimd.tensor_tensor(
                            pt[:], pt[:], bt[:].bitcast(f16), MUL)
                    else:
                        nc.vector.tensor_mul(pt[:], pt[:], bt[:].bitcast(f16))

            def pv_A(i):
                cpc2, st = divmod(i, SKT)
                cp, c2 = divmod(cpc2, 2)
                if st == 0:
                    po_t[cpc2] = [
                        pop.tile([D + 1, 512], f32, tag="po",
                                 name=f"poA{cpc2}_{h}")
                        for h in range(2)]
                po = po_t[cpc2]
                pt = pt_tiles.pop(i)
                for h in range(2):
                    nc.tensor.matmul(
                        po[h][:],
                        vp[:, st * VST + h * 65: st * VST + h * 65 + D + 1],
                        pt[:, h * 512:(h + 1) * 512],
                        start=(st == 0), stop=(st == SKT - 1))
                if st == SKT - 1:
                    for h in range(2):
                        ot = otp.tile([D + 1, 512], f32, tag="ot", name="ot")
                        nc.vector.tensor_copy(ot[:], po[h][:])
                        nc.sync.dma_start(
                            out[h, :, sqoff(cp, c2):sqoff(cp, c2) + 512],
                            ot[:])

            # phase-A injected packet schedule (group index -> packets).
            # k0 chunk c is needed by iteration 4c (group 2c); q0 chunk c by
            # iteration 16c (group 8c); q1/k1 only by phase B.
            def injected(g):
                # spread packets so early groups aren't PE-oversubscribed;
                # q0 chunk c is needed by group 8c, q1/k1 only by phase B.
                if g <= 6:
                    v_proj2(2 * g + 2)
                if g in (0, 1, 2):
                    q0k0_chunk(k0, wk_sb, bk_sb, g + 1, "v")
                elif g == 4:
                    q0k0_chunk(q0, wq_sb, bq_sb, 1, "v")
                elif g == 10:
                    q0k0_chunk(q0, wq_sb, bq_sb, 2, "v")
                elif g == 18:
                    q0k0_chunk(q0, wq_sb, bq_sb, 3, "v")
                elif g in (20, 22, 24, 26):
                    q1k1_chunk((g - 20) // 2, "v")
                elif g == 28:
                    # duplicate head-2 q/k halves for row-tiled self-pairing
                    nc.scalar.dma_start(q1[64:128, :], q1[0:64, :])
                    nc.scalar.dma_start(k1[0:64, :], k1[64:128, :])

            for i in range(PRE):
                bias_dma_A(i)
            for i0 in range(0, NIT_A + LAG, 2):
                for di in range(2):
                    if i0 + di < NIT_A:
                        qk_evac_A(i0 + di)
                injected(i0 // 2)
                for di in range(2):
                    ip = i0 + di - LAG
                    if 0 <= ip < NIT_A:
                        pv_A(ip)

            # ---------- phase B: head 2 (paired with itself) ----------
            bt2_tiles = {}
            pt2_tiles = {}
            po2_t = {}

            def bias_dma_B(i):
                if i >= NIT_B:
                    return
                cpc2, stp = divmod(i, SKT // 2)
                cp, c2 = divmod(cpc2, 2)
                bt = biasp.tile([128, 1024], i16, tag="bt", name=f"btB{i}")
                col = cp * 2048 + c2 * 1024
                nc.sync.dma_start(
                    bt[:], biasB[stp * 128:(stp + 1) * 128, col:col + 1024])
                bt2_tiles[i] = bt

            def qk_evac_B(i):
                cpc2, stp = divmod(i, SKT // 2)
                cp, c2 = divmod(cpc2, 2)
                bias_dma_B(i + PRE)
                sq = sqoff(cp, c2)
                st0, st1 = 2 * stp, 2 * stp + 1
                ps = psp.tile([128, 1024], f32, tag="ps", name=f"psB2_{i}")
                pt = ptp.tile([128, 1024], f16, tag="pt", name=f"ptB{i}")
                pt2_tiles[i] = pt
                nc.tensor.matmul(
                    ps[:, 0:512], k1[0:64, st0 * 128:(st0 + 1) * 128],
                    q1[0:64, sq:sq + 512], start=True, stop=True)
                nc.tensor.matmul(
                    ps[:, 512:1024], k1[64:128, st1 * 128:(st1 + 1) * 128],
                    q1[64:128, sq:sq + 512], start=True, stop=True)
                bt = bt2_tiles.pop(i)
                if _sch_B(i):
                    nc.vector.scalar_tensor_tensor(
                        pt[:].bitcast(i16), ps[:], SCH_OFF,
                        bt[:].bitcast(f16), ADD, ADD)
                else:
                    nc.scalar.activation(pt[:], ps[:], EXP, scale=1.0 / A_EXP)
                    if _gmul_B(i):
                        nc.gpsimd.tensor_tensor(
                            pt[:], pt[:], bt[:].bitcast(f16), MUL)
                    else:
                        nc.vector.tensor_mul(pt[:], pt[:], bt[:].bitcast(f16))

            def pv_B(i):
                cpc2, stp = divmod(i, SKT // 2)
                cp, c2 = divmod(cpc2, 2)
                if stp == 0:
                    po2_t[cpc2] = pop.tile(
                        [D + 1, 512], f32, tag="po", name=f"poB{cpc2}")
                po = po2_t[cpc2]
                pt = pt2_tiles.pop(i)
                for par in range(2):
                    st = 2 * stp + par
                    nc.tensor.matmul(
                        po[:],
                        vp[:, st * VST + 2 * 65: st * VST + 2 * 65 + D + 1],
                        pt[:, par * 512:(par + 1) * 512],
                        start=(stp == 0 and par == 0),
                        stop=(stp == SKT // 2 - 1 and par == 1))
                if stp == SKT // 2 - 1:
                    ot = otp.tile([D + 1, 512], f32, tag="ot", name="ot")
                    nc.vector.tensor_copy(ot[:], po[:])
                    nc.scalar.dma_start(
                        out[2, :, sqoff(cp, c2):sqoff(cp, c2) + 512],
                        ot[:])

            for i in range(PRE):
                bias_dma_B(i)
            for i0 in range(0, NIT_B + LAG, 2):
                for di in range(2):
                    if i0 + di < NIT_B:
                        qk_evac_B(i0 + di)
                for di in range(2):
                    ip = i0 + di - LAG
                    if 0 <= ip < NIT_B:
                        pv_B(ip)

    nc.compile()
    return nc


def _get_nc(variant=None):
    if "nc" not in _CACHE:
        _CACHE["nc"] = _build_nc()
    return _CACHE["nc"]


def _transform_bias(bias_c):
    """Per-tile transformed bias for one core: exp(b) for ScalarE tiles,
    b*1024/ln2 - C_SCH (Schraudolph) for DVE tiles.  bias_c: [3, sk, sq]."""
    tb = np.empty((HPC, S, S), dtype=np.float16)
    for cpc2 in range(4):
        cp, c2 = divmod(cpc2, 2)
        cs = slice(cp * 1024 + c2 * 512, cp * 1024 + c2 * 512 + 512)
        for st in range(SKT):
            i = cpc2 * SKT + st
            rs = slice(st * 128, (st + 1) * 128)
            for h in (0, 1):
                src = bias_c[h][rs, cs]
                if _sch_A(i):
                    tb[h][rs, cs] = (A_EXP * src - C_SCH).astype(np.float16)
                else:
                    tb[h][rs, cs] = np.exp(src).astype(np.float16)
        for stp in range(SKT // 2):
            i = cpc2 * (SKT // 2) + stp
            rs = slice(stp * 256, (stp + 1) * 256)
            src = bias_c[2][rs, cs]
            if _sch_B(i):
                tb[2][rs, cs] = (A_EXP * src - C_SCH).astype(np.float16)
            else:
                tb[2][rs, cs] = np.exp(src).astype(np.float16)
    return tb


def _make_in_maps(hidden_states, Wqkv_w, Wqkv_b, bias, indices, variant=None):
    hidden_states = np.asarray(hidden_states, dtype=np.float32)
    Wqkv_w = np.asarray(Wqkv_w, dtype=np.float32)
    Wqkv_b = np.asarray(Wqkv_b, dtype=np.float32)
    bias = np.asarray(bias, dtype=np.float32)
    indices = np.asarray(indices, dtype=np.int64)

    qscale = np.float32(A_EXP / math.sqrt(D))
    padded = np.zeros((TOTAL, DIM), dtype=np.float32)
    padded[indices] = hidden_states

    Wq, Wk, Wv = Wqkv_w[0:DIM], Wqkv_w[DIM:2 * DIM], Wqkv_w[2 * DIM:3 * DIM]
    bq_full = Wqkv_b[0:DIM] * qscale
    bk_full = Wqkv_b[DIM:2 * DIM]

    def pack_w(WT):  # [768, C] -> [128, 6*C]
        C = WT.shape[1]
        return np.ascontiguousarray(
            WT.reshape(KT, 128, C).transpose(1, 0, 2).reshape(128, KT * C))

    in_maps = []
    for cidx in range(N_CORES):
        b = cidx // 4
        h0 = (cidx % 4) * HPC
        r = slice(h0 * D, (h0 + HPC) * D)
        bias_c = np.ascontiguousarray(bias[b, h0:h0 + HPC].transpose(0, 2, 1))
        tb = _transform_bias(bias_c).view(np.int16)  # [3, sk, sq]
        # biasA: [sk 2048, (cp 2, c2 2, h 2, x 512)]
        bA = tb[0:2].reshape(2, S, 2, 2, 512).transpose(1, 2, 3, 0, 4)
        bA = np.ascontiguousarray(bA.reshape(S, 2 * S))
        # biasB: [stp*128+p, (cp 2, c2 2, par 2, x 512)]
        bB = tb[2].reshape(8, 2, 128, 2, 2, 512).transpose(0, 2, 3, 4, 1, 5)
        bB = np.ascontiguousarray(bB.reshape(S // 2, 2 * S))
        bqk_arr = np.zeros((128, 4), dtype=np.float32)
        bqk_arr[:, 0] = bq_full[r][0:128]
        bqk_arr[0:64, 1] = bq_full[r][128:192]
        bqk_arr[:, 2] = bk_full[r][0:128]
        bqk_arr[64:128, 3] = bk_full[r][128:192]
        WqT = Wq[r].T * qscale
        WkT = Wk[r].T
        in_maps.append({
            "hT": padded[b * S:(b + 1) * S].T.astype(np.float16),
            "wq": pack_w(WqT[:, 0:128].astype(np.float16)),
            "wk": pack_w(WkT[:, 0:128].astype(np.float16)),
            "wqk1": pack_w(np.concatenate(
                [WqT[:, 128:192], WkT[:, 128:192]], axis=1).astype(np.float16)),
            "wv": pack_w(Wv[r].T.astype(np.float16)),
            "bqk": bqk_arr,
            "biasA": bA,
            "biasB": bB,
        })
    return in_maps


def _assemble(results, Wqkv_b, indices):
    Wqkv_b = np.asarray(Wqkv_b, dtype=np.float32)
    indices = np.asarray(indices, dtype=np.int64)
    bv = Wqkv_b[2 * DIM:3 * DIM]
    out_full = np.empty((TOTAL, DIM), dtype=np.float32)
    for c in range(N_CORES):
        b = c // 4
        h0 = (c % 4) * HPC
        o = np.asarray(results[c]["out"], dtype=np.float32)  # (3, 65, 2048)
        for j in range(HPC):
            h = h0 + j
            att = (o[j, :D] / o[j, D]).T + bv[h * D:(h + 1) * D]
            out_full[b * S:(b + 1) * S, h * D:(h + 1) * D] = att
    return out_full[indices]


VARIANT = "v6"


def kernel(hidden_states, Wqkv_w, Wqkv_b, bias, slopes, cu_seqlens, indices,
           attn_mask, max_seqlen, **_unused):
    from concourse.bass_utils import run_bass_kernel_spmd

    nc = _get_nc()
    in_maps = _make_in_maps(hidden_states, Wqkv_w, Wqkv_b, bias, indices)
    res = run_bass_kernel_spmd(nc, in_maps, list(range(N_CORES)))
    return _assemble(res.results, Wqkv_b, indices)
